# revision 1
# baseline (speedup 1.0000x reference)
"""Trainium2 Bass kernel for nn_EquivariantConvolutionBlock (sparse 5^3 equivariant
conv + gate + batchnorm over 300k voxels in a 128^3 grid), SPMD over 8 NeuronCores.

Strategy (per core = one x-slab of 16 grid planes, halo +-2):
- Host folds the e3nn tensor-product kernel + self-connection into per-window
  stationary matrices [128,112] (4 stencil z-offsets x 32 ch contracted per matmul).
- Host builds a fp16 "neighborhood block" volume B: each 1KB row = a [4dy x 4dz]
  block of 32-ch features (sliding windows in both y and z), so ONE dma_gather
  element feeds FOUR K=128 matmul columns (4 dy values x 4 dz slots each).
- 8 window-blocks per voxel cover the whole active 5^3 stencil (81 offsets incl.
  the folded self-connection). dma_gather (int16 idx, static per-plane base,
  4 SWDGE queues) builds the rhs; PSUM accumulates 30 nonzero block-column
  matmuls per 512-voxel tile; sigmoid/gate on ACT/DVE; BatchNorm batch stats
  AllReduce'd across cores; normalization applied in a second streaming pass.
"""
import sys

sys.path.insert(0, "/opt/trn_rl_repo")

import os
import numpy as np
from contextlib import ExitStack

import concourse.bass as bass
import concourse.bacc as bacc
import concourse.tile as tile
import concourse.mybir as mybir
from concourse.bass_utils import run_bass_kernel_spmd

F16 = mybir.dt.float16
F32 = mybir.dt.float32
I16 = mybir.dt.int16

N = 300000
GRID = 128
NCORES = 8
EPS = 1e-5
PPC = int(os.environ.get("KPPC", "16"))  # planes per core
NOCC = os.environ.get("NOCC", "0") == "1"
KSTAGE = int(os.environ.get("KSTAGE", "3"))
YQ = 132          # y-block-start axis
SD = 132          # z-window-start axis
PLANE_ROWS = SD * YQ          # 17424
BROWS = 20 * PLANE_ROWS       # B-volume rows per core (20 x-planes incl halo)
TCOLS = 512
PAD_IDX = 130     # (s=0, yq=130): all-zero elem

# window-blocks: (dx, ady, adz); block covers dy in [ady, ady+3], dz in [adz, adz+3]
WBS = [(-2, -1, -1),
       (-1, -2, -2), (-1, -1, -1),
       (0, -2, -2), (0, -1, -1),
       (1, -2, -2), (1, -1, -1),
       (2, -1, -1)]

_COMPILED = None


# ---------------------------------------------------------------- host math

def _soft_unit_step(t):
    out = np.zeros_like(t)
    m = t > 0
    out[m] = np.exp(-1.0 / t[m])
    return out


def _make_ker_by_off(tp_weight, Ws1, Ws2, Wv):
    ax = np.arange(-2, 3.0)
    lat = np.stack(np.meshgrid(ax, ax, ax, indexing="ij"), -1).reshape(-1, 3)
    d = np.linalg.norm(lat, axis=-1)
    values = np.linspace(0.0, 2.5, 5)[1:-1]
    step = 2.5 / 4
    diff = (d[..., None] - values) / step
    emb = 1.14136 * float(np.e ** 2) * _soft_unit_step(diff + 1.0) * _soft_unit_step(1.0 - diff)
    w = (emb @ tp_weight.astype(np.float64)) / 125.0
    w1, w2, w3, w4, w5, w6 = [w[:, i * 128:(i + 1) * 128].reshape(-1, 8, 16) for i in range(6)]
    unit = np.where(d[:, None] > 0, lat / np.where(d > 0, d, 1.0)[:, None], 0.0)
    y1 = np.sqrt(3.0) * unit
    A = 0.25
    B = A / np.sqrt(3.0)
    Cc = 0.25
    M_ss = A * w1
    M_vs = (B * np.einsum("xi,xuw->xuiw", y1, w2)).reshape(-1, 24, 16)
    M_sg = A * w3
    M_vg = (B * np.einsum("xi,xuw->xuiw", y1, w4)).reshape(-1, 24, 16)
    M_sv = (Cc * np.einsum("xk,xuw->xuwk", y1, w5)).reshape(-1, 8, 48)
    M_vv = (Cc * np.einsum("xuw,ik->xuiwk", w6, np.eye(3))).reshape(-1, 24, 48)
    top = np.concatenate([M_ss, M_sg, M_sv], -1)
    bot = np.concatenate([M_vs, M_vg, M_vv], -1)
    ker = np.concatenate([top, bot], 1)            # [125,32,80]
    kbo = {tuple(int(v) for v in lat[i]): ker[i] for i in range(125)}
    inv = 1.0 / np.sqrt(8.0)
    Wsc = np.zeros((32, 80))
    Wsc[0:8, 0:16] = Ws1 * inv
    Wsc[0:8, 16:32] = Ws2 * inv
    u, w_ = np.meshgrid(np.arange(8), np.arange(16), indexing="ij")
    for i in range(3):
        Wsc[8 + u * 3 + i, 32 + w_ * 3 + i] = Wv * inv
    kbo[(0, 0, 0)] = kbo[(0, 0, 0)] + Wsc          # emb(0)=0, so center slot is free
    return kbo


def _active(dx, dy, dz):
    d2 = dx * dx + dy * dy + dz * dz
    return (0 < d2 <= 6) or (dx, dy, dz) == (0, 0, 0)


def _assigned_wb(dx, dy, dz):
    """Index into WBS owning cell (dx,dy,dz); each active cell owned once."""
    for i, (wdx, ady, adz) in enumerate(WBS):
        if wdx == dx and ady <= dy <= ady + 3 and adz <= dz <= adz + 3:
            return i
    raise AssertionError((dx, dy, dz))


def _nonzero_slots():
    nz = []
    for wbi, (dx, ady, adz) in enumerate(WBS):
        for q in range(4):
            dy = ady + q
            for k in range(4):
                dz = adz + k
                if (abs(dy) <= 2 and abs(dz) <= 2 and _active(dx, dy, dz)
                        and _assigned_wb(dx, dy, dz) == wbi):
                    nz.append((wbi, q))
                    break
    return nz


def _build_stationaries(kbo):
    """[128, 32*112] f16: slot (wbi*4+q) = column (dx, ady+q), rows 32k+c = dz=adz+k.
    Output channel layout: 0:16 s | 32:48 gates | 64:112 v."""
    kers = np.zeros((128, len(WBS) * 4 * 112), np.float32)
    for wbi, (dx, ady, adz) in enumerate(WBS):
        for q in range(4):
            dy = ady + q
            c0 = (wbi * 4 + q) * 112
            for k in range(4):
                dz = adz + k
                if abs(dy) > 2 or abs(dz) > 2 or not _active(dx, dy, dz):
                    continue
                if _assigned_wb(dx, dy, dz) != wbi:
                    continue
                m = kbo[(dx, dy, dz)]
                kers[32 * k:32 * (k + 1), c0 + 0:c0 + 16] = m[:, 0:16]
                kers[32 * k:32 * (k + 1), c0 + 32:c0 + 48] = m[:, 16:32]
                kers[32 * k:32 * (k + 1), c0 + 64:c0 + 112] = m[:, 32:80]
    return kers.astype(np.float16)


def _wrap_idx(flat):
    w16 = flat.reshape(-1, 16).T.astype(np.int16)
    return np.tile(w16, (8, 1))


# ---------------------------------------------------------------- device program

def _build_program(TPP):
    PCOLS = TPP * TCOLS
    nzset = set(_nonzero_slots())
    NTILE = PPC * TPP
    NPC = PPC * PCOLS
    IDXW = PPC * len(WBS) * PCOLS // 16

    nc = bacc.Bacc("TRN2", target_bir_lowering=False, debug=False,
                   num_devices=NCORES, num_swdge_queues=4)
    B_t = nc.dram_tensor("bvol", [BROWS, 512], F16, kind="ExternalInput").ap()
    IDX_t = nc.dram_tensor("idx", [128, IDXW], I16, kind="ExternalInput").ap()
    KER_t = nc.dram_tensor("kers", [128, len(WBS) * 4 * 112], F16,
                           kind="ExternalInput").ap()
    E_t = nc.dram_tensor("emat", [16, 48], F32, kind="ExternalInput").ap()
    BN_t = nc.dram_tensor("bn", [1, 48], F32, kind="ExternalInput").ap()
    OUT_t = nc.dram_tensor("out", [64, NPC], F32, kind="ExternalOutput").ap()

    gather_ct = 0

    with tile.TileContext(nc) as tc, ExitStack() as ctx:
        cpool = ctx.enter_context(tc.tile_pool(name="const", bufs=1))
        ipool = ctx.enter_context(tc.tile_pool(name="idxp", bufs=8))
        rpool = ctx.enter_context(tc.tile_pool(name="rhs", bufs=8))
        spool = ctx.enter_context(tc.tile_pool(name="small", bufs=3))
        qpool = ctx.enter_context(tc.tile_pool(name="sq", bufs=2))
        vpool = ctx.enter_context(tc.tile_pool(name="sv", bufs=4))
        opool = ctx.enter_context(tc.tile_pool(name="outp", bufs=2))
        stpool = ctx.enter_context(tc.tile_pool(name="stats", bufs=1))
        pp = ctx.enter_context(tc.tile_pool(name="psA", bufs=6, space="PSUM"))
        pg = ctx.enter_context(tc.tile_pool(name="psB", bufs=2, space="PSUM"))
        dpool = ctx.enter_context(tc.tile_pool(name="dram", bufs=1, space="DRAM"))

        kers = cpool.tile([128, len(WBS) * 4 * 112], F16)
        nc.sync.dma_start(kers[:], KER_t[:])
        e48 = cpool.tile([48, 48], F32)
        nc.sync.dma_start(e48[32:48, :], E_t[:])
        bn = cpool.tile([1, 48], F32)
        nc.sync.dma_start(bn[:], BN_t[:])
        ones = cpool.tile([1, 1], F32)
        nc.vector.memset(ones[:], 1.0)

        stats_s = stpool.tile([16, NTILE], F32, tag="sts")
        stats_s2 = stpool.tile([16, NTILE], F32, tag="sts2")
        stats_v2 = stpool.tile([112, NTILE], F32, tag="stv2")

        svd = dpool.tile([64, NPC], F16)            # DRAM spill of gated pre-BN s|v

        nz_by_wb = {}
        for wbi, q in sorted(nzset):
            nz_by_wb.setdefault(wbi, []).append(q)
        wb_order = sorted(nz_by_wb.keys())
        first_wb, last_wb = wb_order[0], wb_order[-1]

        for j in range(PPC):
            ps = [pp.tile([112, TCOLS], F32, tag="convps", name=f"ps_{j}_{k}")
                  for k in range(TPP)]
            for wbi in wb_order:
                dx, ady, adz = WBS[wbi]
                base = (j + 2 + dx) * PLANE_ROWS
                qs = nz_by_wb[wbi]
                for sub in range(TPP):
                    off16 = ((j * len(WBS) + wbi) * PCOLS + sub * TCOLS) // 16
                    idxt = ipool.tile([128, TCOLS // 16], I16, tag="idx",
                                      name=f"ix_{j}_{wbi}_{sub}")
                    nc.sync.dma_start(idxt[:], IDX_t[:, off16: off16 + TCOLS // 16])
                    rhs = rpool.tile([128, 4, TCOLS], F16, tag="rhs",
                                     name=f"rh_{j}_{wbi}_{sub}")
                    nc.gpsimd.dma_gather(
                        rhs[:], B_t[base: base + PLANE_ROWS, :], idxt[:],
                        TCOLS, TCOLS, 512, transpose=True,
                        queue_num=gather_ct % 4,
                    )
                    gather_ct += 1
                    for q in qs:
                        nc.tensor.matmul(
                            ps[sub][:],
                            kers[:, (wbi * 4 + q) * 112:(wbi * 4 + q + 1) * 112],
                            rhs[:, q, :],
                            start=(wbi == first_wb and q == qs[0]),
                            stop=(wbi == last_wb and q == qs[-1]),
                        )
            if KSTAGE < 2:
                for sub in range(TPP):
                    ti = j * TPP + sub
                    oconv = opool.tile([64, TCOLS], F32, tag="oconv")
                    nc.scalar.activation(oconv[:], ps[sub][0:64, :],
                                         mybir.ActivationFunctionType.Copy)
                    nc.sync.dma_start(OUT_t[:, ti * TCOLS:(ti + 1) * TCOLS], oconv[:])
                continue
            # post-processing per 512-col tile
            for sub in range(TPP):
                ti = j * TPP + sub
                sig = spool.tile([48, TCOLS], F32, tag="sig")
                nc.scalar.activation(sig[:], ps[sub][0:48, :],
                                     mybir.ActivationFunctionType.Sigmoid)
                gex = pg.tile([112, TCOLS], F32, tag="gexps")
                nc.tensor.matmul(gex[64:112, :], e48[32:48, :], sig[32:48, :],
                                 start=True, stop=True, tile_position=(32, 64))
                svs = vpool.tile([16, TCOLS], F16, tag="svs")
                nc.vector.tensor_tensor(svs[:], ps[sub][0:16, :], sig[0:16, :],
                                        mybir.AluOpType.mult)
                gexs = spool.tile([112, TCOLS], F32, tag="gexs")
                nc.scalar.activation(gexs[64:112, :], gex[64:112, :],
                                     mybir.ActivationFunctionType.Copy)
                svv = vpool.tile([112, TCOLS], F16, tag="svv")
                nc.vector.tensor_tensor(svv[64:112, :], ps[sub][64:112, :],
                                        gexs[64:112, :], mybir.AluOpType.mult)
                if KSTAGE >= 3:
                    nc.vector.tensor_reduce(stats_s[:, ti:ti + 1], svs[:],
                                            mybir.AxisListType.X, mybir.AluOpType.add)
                    sq1 = qpool.tile([16, TCOLS], F32, tag="sq1")
                    nc.scalar.activation(sq1[:], svs[:],
                                         mybir.ActivationFunctionType.Square,
                                         accum_out=stats_s2[:, ti:ti + 1])
                    sq2 = qpool.tile([112, TCOLS], F32, tag="sq2")
                    nc.scalar.activation(sq2[64:112, :], svv[64:112, :],
                                         mybir.ActivationFunctionType.Square,
                                         accum_out=stats_v2[64:112, ti:ti + 1])
                nc.sync.dma_start(svd[0:16, ti * TCOLS:(ti + 1) * TCOLS], svs[:])
                nc.sync.dma_start(svd[16:64, ti * TCOLS:(ti + 1) * TCOLS],
                                  svv[64:112, :])

        # ---- batch statistics: reduce partials, AllReduce, finalize scales
        if KSTAGE == 2:
            a_col = stpool.tile([64, 1], F32, tag="acol")
            nc.vector.memset(a_col[:], 1.0)
            b_col = stpool.tile([64, 1], F32, tag="bcol")
            nc.vector.memset(b_col[:], 0.0)
        if KSTAGE >= 3:
            red_s = stpool.tile([16, 1], F32)
            nc.vector.tensor_reduce(red_s[:], stats_s[:], mybir.AxisListType.X,
                                    mybir.AluOpType.add)
            red_s2 = stpool.tile([16, 1], F32)
            nc.vector.tensor_reduce(red_s2[:], stats_s2[:], mybir.AxisListType.X,
                                    mybir.AluOpType.add)
            red_v2 = stpool.tile([112, 1], F32)
            nc.vector.tensor_reduce(red_v2[64:112, :], stats_v2[64:112, :],
                                    mybir.AxisListType.X, mybir.AluOpType.add)
            cc_in = dpool.tile([1, 80], F32)
            cc_out = dpool.tile([1, 80], F32)
            nc.sync.dma_start(cc_in[0:1, 0:16], red_s[:])
            nc.sync.dma_start(cc_in[0:1, 16:32], red_s2[:])
            nc.sync.dma_start(cc_in[0:1, 32:80], red_v2[64:112, :])
            if not NOCC:
                nc.gpsimd.collective_compute(
                    "AllReduce", mybir.AluOpType.add,
                    replica_groups=[list(range(NCORES))],
                    ins=[cc_in.opt()], outs=[cc_out.opt()],
                )
            st = stpool.tile([1, 80], F32)
            nc.sync.dma_start(st[:], cc_in[:] if NOCC else cc_out[:])

            def rsqrt16(dst, src_ap, sfx):
                t = stpool.tile([1, 16], F32, tag=f"nt_{sfx}")
                nc.vector.tensor_scalar_add(t[:], src_ap, EPS)
                r = stpool.tile([1, 16], F32, tag=f"nr_{sfx}")
                nc.vector.reciprocal(r[:], t[:])
                q = stpool.tile([1, 16], F32, tag=f"nq_{sfx}")
                nc.scalar.activation(q[:], r[:], mybir.ActivationFunctionType.Sqrt)
                qq = stpool.tile([1, 16], F32, tag=f"nqq_{sfx}")
                nc.vector.tensor_tensor(qq[:], q[:], q[:], mybir.AluOpType.mult)
                nc.vector.tensor_tensor(qq[:], qq[:], t[:], mybir.AluOpType.mult)
                nc.vector.tensor_scalar_mul(qq[:], qq[:], -0.5)
                nc.vector.tensor_scalar_add(qq[:], qq[:], 1.5)
                nc.vector.tensor_tensor(dst[:], q[:], qq[:], mybir.AluOpType.mult)

            mu = stpool.tile([1, 16], F32)
            nc.vector.tensor_scalar_mul(mu[:], st[0:1, 0:16], 1.0 / N)
            var = stpool.tile([1, 16], F32)
            nc.vector.tensor_scalar_mul(var[:], st[0:1, 16:32], 1.0 / N)
            mumu = stpool.tile([1, 16], F32)
            nc.vector.tensor_tensor(mumu[:], mu[:], mu[:], mybir.AluOpType.mult)
            nc.vector.tensor_tensor(var[:], var[:], mumu[:], mybir.AluOpType.subtract)
            a_s = stpool.tile([1, 16], F32)
            rsqrt16(a_s, var[:], "s")
            nc.vector.tensor_tensor(a_s[:], a_s[:], bn[0:1, 0:16],
                                    mybir.AluOpType.mult)
            b_s = stpool.tile([1, 16], F32)
            nc.vector.tensor_tensor(b_s[:], mu[:], a_s[:], mybir.AluOpType.mult)
            nc.vector.tensor_tensor(b_s[:], bn[0:1, 32:48], b_s[:],
                                    mybir.AluOpType.subtract)
            vn = stpool.tile([1, 16], F32)
            v3 = st[0:1, 32:80].rearrange("p (g d) -> p g d", d=3)
            nc.vector.tensor_reduce(vn[:], v3, mybir.AxisListType.X,
                                    mybir.AluOpType.add)
            nc.vector.tensor_scalar_mul(vn[:], vn[:], 1.0 / (3.0 * N))
            a_v = stpool.tile([1, 16], F32)
            rsqrt16(a_v, vn[:], "v")
            nc.vector.tensor_tensor(a_v[:], a_v[:], bn[0:1, 16:32],
                                    mybir.AluOpType.mult)
            a_vec = stpool.tile([1, 64], F32)
            nc.vector.tensor_copy(a_vec[0:1, 0:16], a_s[:])
            av3 = a_vec[0:1, 16:64].rearrange("p (g d) -> p g d", d=3)
            avs = a_v[0:1, :].rearrange("p (g d) -> p g d", d=1)
            for i in range(3):
                nc.vector.tensor_copy(av3[:, :, i:i + 1], avs[:])
            b_vec = stpool.tile([1, 64], F32)
            nc.vector.memset(b_vec[:], 0.0)
            nc.vector.tensor_copy(b_vec[0:1, 0:16], b_s[:])
            abps = pg.tile([64, 1], F32, tag="gexps")
            nc.tensor.matmul(abps[:], a_vec[:], ones[:], start=True, stop=True)
            a_col = stpool.tile([64, 1], F32)
            nc.scalar.activation(a_col[:], abps[:], mybir.ActivationFunctionType.Copy)
            abps2 = pg.tile([64, 1], F32, tag="gexps")
            nc.tensor.matmul(abps2[:], b_vec[:], ones[:], start=True, stop=True)
            b_col = stpool.tile([64, 1], F32)
            nc.scalar.activation(b_col[:], abps2[:], mybir.ActivationFunctionType.Copy)

        # ---- apply normalization: out = sv * a + b
        for j in range(PPC if KSTAGE >= 2 else 0):
            svt = opool.tile([64, PCOLS], F16, tag="svin")
            nc.sync.dma_start(svt[:], svd[:, j * PCOLS:(j + 1) * PCOLS])
            outt = opool.tile([64, PCOLS], F32, tag="outt")
            nc.scalar.activation(outt[:], svt[:],
                                 mybir.ActivationFunctionType.Identity,
                                 bias=b_col[:], scale=a_col[:])
            nc.sync.dma_start(OUT_t[:, j * PCOLS:(j + 1) * PCOLS], outt[:])

    nc.compile()
    return nc


# ---------------------------------------------------------------- host driver

def _prep_inputs(inputs, TPP):
    PCOLS = TPP * TCOLS
    feats = np.asarray(inputs["feats"], np.float32)
    coords = np.asarray(inputs["coords"], np.int64)
    kbo = _make_ker_by_off(np.asarray(inputs["tp_weight"], np.float64),
                           np.asarray(inputs["Ws1"], np.float64),
                           np.asarray(inputs["Ws2"], np.float64),
                           np.asarray(inputs["Wv"], np.float64))
    kers = _build_stationaries(kbo)
    E = np.zeros((16, 48), np.float32)
    for w in range(16):
        for i in range(3):
            E[w, w * 3 + i] = 1.0
    BN = np.concatenate([np.asarray(inputs["bn_weight"], np.float32),
                         np.asarray(inputs["bn_bias"], np.float32)])[None, :]

    lin = (coords[:, 0] * GRID + coords[:, 1]) * GRID + coords[:, 2]
    perm = np.argsort(lin, kind="stable")
    cs = coords[perm]
    fs = feats[perm].astype(np.float16)

    from numpy.lib.stride_tricks import sliding_window_view
    in_maps = []
    counts = np.zeros((NCORES, PPC), np.int64)
    for c in range(NCORES):
        x0 = 16 * c
        m = (cs[:, 0] >= x0 - 2) & (cs[:, 0] < x0 + 18)
        cc, ff = cs[m], fs[m]
        # V slots: [20 xp, 136 yp(pad), 136 zp] of 32 fp16
        Vs = np.zeros((20, 136, 136, 32), np.float16)
        Vs[cc[:, 0] - x0 + 2, cc[:, 1] + 2, cc[:, 2] + 2] = ff
        # A4[xp, yp, s] = Vs[xp, yp, s:s+4]  -> [20,136,132,128]
        A4 = sliding_window_view(Vs, 4, axis=2)[:, :, :SD]     # [20,136,132,32,4]
        A4 = np.ascontiguousarray(A4.transpose(0, 1, 2, 4, 3)).reshape(20, 136, SD, 128)
        # B[xp, s, yq] = concat_k A4[xp, yq+k, s]  -> [BROWS, 512]
        sw = sliding_window_view(A4, 4, axis=1)                # [20,133,132,128,4]
        Bv = np.ascontiguousarray(
            sw[:, :YQ].transpose(0, 2, 1, 4, 3)).reshape(BROWS, 512)

        mloc = (cs[:, 0] >= x0) & (cs[:, 0] < x0 + 16)
        cl = cs[mloc]
        idx_blocks = []
        for j in range(PPC):
            pm = cl[:, 0] == x0 + j
            y, z = cl[pm, 1], cl[pm, 2]
            n = len(y)
            assert n <= PCOLS, f"plane overflow {n} > {PCOLS}"
            counts[c, j] = n
            for (dx, ady, adz) in WBS:
                blk = np.full(PCOLS, PAD_IDX, np.int64)
                blk[:n] = (z + 2 + adz) * YQ + (y + 2 + ady)
                idx_blocks.append(blk)
        idx = _wrap_idx(np.concatenate(idx_blocks))
        in_maps.append({"bvol": Bv, "idx": idx, "kers": kers,
                        "emat": E, "bn": BN})
    return in_maps, counts, perm, cs


def kernel(**inputs):
    global _COMPILED
    coords = np.asarray(inputs["coords"], np.int64)
    maxp = int(np.bincount(coords[:, 0], minlength=GRID).max())
    TPP = max(5, -(-maxp // TCOLS))
    if _COMPILED is None or _COMPILED[0] != TPP:
        nc = _build_program(TPP)
        _COMPILED = (TPP, nc)
    else:
        nc = _COMPILED[1]
    PCOLS = TPP * TCOLS
    in_maps, counts, perm, cs = _prep_inputs(inputs, TPP)
    res = run_bass_kernel_spmd(nc, in_maps, core_ids=list(range(NCORES)))
    pieces = []
    for c in range(NCORES):
        o = res.results[c]["out"]
        for j in range(PPC):
            n = counts[c, j]
            if n:
                pieces.append(o[:, j * PCOLS: j * PCOLS + n])
    sorted_out = np.concatenate(pieces, axis=1).T
    out = np.empty_like(sorted_out)
    out[perm] = sorted_out
    return out

